# revision 24
# baseline (speedup 1.0000x reference)
"""Trainium2 Bass kernel for sliding-window multi-head attention with qk-norm.

Problem (hardcoded): B=2, S=2048, E=1024, H=16, D=64, WINDOW=512, fp32.

Sharding: heads across 8 cores (2 heads/core, all tokens), AllToAll of head
outputs, token-split out-projection (512 tokens/core).

v3 design notes:
- LN mean-subtraction folded into W_qkv columns host-side; gamma product
  folded into k-side weights; variance via gamma-weighted selector matmul.
- rstd = Exp(-0.5*Ln(var+eps)); the activation-table registry is patched
  so the whole kernel uses ONE table set (natural_log_exp_and_others).
- Attention is key-block stationary: per key block j and head h, one
  [128,<=512] span-score matmul (queries ks+128..ks+640) + one [128,128]
  diagonal matmul share a [128,640] PSUM tile and one 640-wide Exp call
  with the per-key 1/(8*sigma_k) factor in the activation scale AP.
- The two heads' chains are interleaved block-by-block so the tensor
  engine always has independent work (keeps HAM at full clock).
- A*V accumulates into a rolling [65,1024] PSUM window per head; row 64
  carries the softmax denominator via a ones-column in vhat. Every 4
  blocks one 512-query bank is evacuated unnormalized (+f32 rowsum).
- Normalization is deferred through a single merged AllToAll; the
  token-split side normalizes before the out-projection.
"""

import sys

sys.path.insert(0, "/opt/trn_rl_repo")

import numpy as np
import ml_dtypes

import concourse.bass as bass
import concourse.mybir as mybir
import concourse.tile as tile
from concourse import bacc
from concourse.bass_utils import run_bass_kernel_spmd

F32 = mybir.dt.float32
F32R = mybir.dt.float32r
BF16 = mybir.dt.bfloat16
AF = mybir.ActivationFunctionType

B, S, E, H = 2, 2048, 1024, 16
D = E // H  # 64
WINDOW = 512
EPS = 1e-5
LN8 = float(np.log(8.0))
N_CORES = 8
HPC = H // N_CORES  # heads per core = 2
TOK = B * S  # 4096
CHUNK = 512  # token chunk for projection phase
CPB = 4  # chunks per batch
NBLK = S // 128  # 16 key blocks per batch


def _patch_act_tables(arch):
    """Restrict the activation-table registry to the one set containing
    both Ln and Exp, so the compiler never alternates table loads."""
    from concourse.hw_specs import get_activation_tables

    tabs = get_activation_tables(arch)
    keep = "natural_log_exp_and_others"
    assert keep in tabs, list(tabs)
    for name, fns in tabs.items():
        if name != keep:
            fns.clear()


def build_program():
    nc = bacc.Bacc("TRN2", target_bir_lowering=False, debug=False,
                   num_devices=N_CORES)

    # ---- dram parameters (per-core inputs) ----
    xT = nc.declare_dram_parameter("xT", [E, TOK], BF16, isOutput=False)
    wqkv = nc.declare_dram_parameter("wqkv", [E, 3 * 128], BF16, isOutput=False)
    bqkv = nc.declare_dram_parameter("bqkv", [128, 3], F32, isOutput=False)
    wout = nc.declare_dram_parameter("wout", [E, E], BF16, isOutput=False)
    bout = nc.declare_dram_parameter("bout", [128, 8], F32, isOutput=False)
    mlead = nc.declare_dram_parameter("mlead", [128, 128], BF16, isOutput=False)
    mtrail = nc.declare_dram_parameter("mtrail", [128, 128], BF16, isOutput=False)
    sel2q = nc.declare_dram_parameter("sel2q", [128, 2], BF16, isOutput=False)
    sel2k = nc.declare_dram_parameter("sel2k", [128, 2], BF16, isOutput=False)
    expd = nc.declare_dram_parameter("expd", [2, 128], F32R, isOutput=False)
    identb = nc.declare_dram_parameter("identb", [128, 128], BF16, isOutput=False)
    sel01 = nc.declare_dram_parameter("sel01", [4, 256], F32R, isOutput=False)
    outT = nc.declare_dram_parameter("outT", [E, 512], F32, isOutput=True)

    with tile.TileContext(nc) as tc:
        with (
            nc.allow_low_precision(reason="bf16 matmul pipeline"),
            tc.tile_pool(name="const", bufs=1) as cpool,
            tc.tile_pool(name="persist", bufs=1) as ppool,
            tc.tile_pool(name="xp", bufs=3) as xpool,
            tc.tile_pool(name="tmp", bufs=6) as tpool,
            tc.tile_pool(name="expp", bufs=4) as epool,
            tc.tile_pool(name="hp", bufs=4) as hpool,
            tc.tile_pool(name="outp", bufs=2) as opool,
            tc.tile_pool(name="ps_sc", bufs=2, space="PSUM") as ps_sc,
            tc.tile_pool(name="dram", bufs=1, space="DRAM") as dpool,
        ):
            # ---- internal dram: per-batch collectives (unused slots zero) --
            partb = [dpool.tile([N_CORES, 132, 512], BF16, tag=f"part{b}",
                     name=f"part{b}") for b in range(B)]
            a2ab = [dpool.tile([N_CORES, 132, 512], BF16, tag=f"a2a{b}",
                    name=f"a2a{b}") for b in range(B)]

            # ---- constants group A (needed for projection) ----
            wqkv_sb = cpool.tile([128, 8, 384], BF16)
            nc.sync.dma_start(out=wqkv_sb,
                              in_=wqkv[:, :].rearrange("(t p) c -> p t c", p=128))
            bqkv_sb = cpool.tile([128, 3], F32)
            nc.sync.dma_start(out=bqkv_sb, in_=bqkv[:, :])
            sel2q_sb = cpool.tile([128, 2], BF16)
            nc.sync.dma_start(out=sel2q_sb, in_=sel2q[:, :])
            sel2k_sb = cpool.tile([128, 2], BF16)
            nc.sync.dma_start(out=sel2k_sb, in_=sel2k[:, :])
            expd_sb = cpool.tile([2, 128], F32R)
            nc.sync.dma_start(out=expd_sb, in_=expd[:, :])
            identb_sb = cpool.tile([128, 128], BF16)
            nc.sync.dma_start(out=identb_sb, in_=identb[:, :])
            # per-partition constants: col 0 = EPS, col 1 = -ln(8)
            cc_sb = cpool.tile([128, 2], F32)
            nc.vector.memset(cc_sb[:, 0:1], EPS)
            nc.vector.memset(cc_sb[:, 1:2], -LN8)
            zz_sb = cpool.tile([128, 512], BF16)
            nc.vector.memset(zz_sb[:], 0.0)

            # ---- persistent per-batch tensors ----
            qc = [ppool.tile([128, S], BF16, tag=f"qc{b}", name=f"qc{b}")
                  for b in range(B)]
            kc = [ppool.tile([128, S], BF16, tag=f"kc{b}", name=f"kc{b}")
                  for b in range(B)]
            vhat = [ppool.tile([128, NBLK, 130], BF16, tag=f"vh{b}",
                    name=f"vh{b}") for b in range(B)]
            rk_sb = [ppool.tile([128, NBLK, HPC], F32, tag=f"rk{b}",
                     name=f"rk{b}") for b in range(B)]

            stg = [ppool.tile([128, 8, 512], BF16, tag=f"stg{b}",
                   name=f"stg{b}") for b in range(B)]
            rstg = [ppool.tile([4, 8, 512], BF16, tag=f"rstg{b}",
                    name=f"rstg{b}") for b in range(B)]

            for b in range(B):
                nc.vector.memset(vhat[b][:, :, 64:65].bitcast(mybir.dt.uint16),
                                 0x3F80)
                nc.vector.memset(vhat[b][:, :, 129:130].bitcast(mybir.dt.uint16),
                                 0x3F80)

            # ================= projection for one 512-token chunk ============
            def proj_chunk(b, tci, ps_st):
                t = b * CPB + tci
                ts = tci * CHUNK  # token offset within batch
                xt = xpool.tile([128, 8, CHUNK], BF16, tag="xt")
                nc.sync.dma_start(
                    out=xt,
                    in_=xT[:, :].rearrange("(e p) w -> p e w", p=128)[
                        :, :, t * CHUNK:(t + 1) * CHUNK],
                )
                for c3 in range(3):  # 0=q, 1=k, 2=v
                    mm = ps_sc.tile([128, CHUNK], F32, tag="sc")
                    for et in range(8):
                        nc.tensor.matmul(
                            mm[:],
                            wqkv_sb[:, et, c3 * 128:(c3 + 1) * 128],
                            xt[:, et, :],
                            start=(et == 0),
                            stop=(et == 7),
                        )
                    if c3 == 0:
                        # q: centered by weight prep; scale columns by rq
                        xsb = tpool.tile([128, CHUNK], BF16, tag="xsb")
                        nc.scalar.activation(xsb[:], mm[:], AF.Identity,
                                             bias=bqkv_sb[:, 0:1])
                        sq = tpool.tile([128, CHUNK], BF16, tag="sq")
                        nc.vector.tensor_mul(sq[:], xsb[:], xsb[:])
                        var = ps_st.tile([2, CHUNK], F32, tag="st")
                        nc.tensor.matmul(var[:], sel2q_sb[:], sq[:],
                                         start=True, stop=True)
                        lnv = tpool.tile([2, CHUNK], F32, tag="lnv")
                        nc.scalar.activation(lnv[:], var[:], AF.Ln,
                                             bias=cc_sb[0:2, 0:1])
                        rq = tpool.tile([2, CHUNK], F32R, tag="rq")
                        nc.scalar.activation(rq[:], lnv[:], AF.Exp, scale=-0.5)
                        rbc = ps_st.tile([128, CHUNK], F32, tag="st")
                        nc.tensor.matmul(rbc[:], expd_sb[:], rq[:],
                                         start=True, stop=True)
                        nc.vector.tensor_mul(qc[b][:, ts:ts + CHUNK],
                                             xsb[:], rbc[:])
                    elif c3 == 1:
                        # k: centered+gamma-scaled by weight prep; rk via
                        # gamma-weighted variance, applied in the exp scale
                        nc.scalar.activation(kc[b][:, ts:ts + CHUNK], mm[:],
                                             AF.Identity, bias=bqkv_sb[:, 1:2])
                        sq = tpool.tile([128, CHUNK], BF16, tag="sq")
                        nc.vector.tensor_mul(sq[:], kc[b][:, ts:ts + CHUNK],
                                             kc[b][:, ts:ts + CHUNK])
                        rkv = ps_st.tile([128, 8], F32, tag="st")
                        for jj in range(4):
                            nc.tensor.matmul(
                                rkv[:, 2 * jj:2 * jj + 2],
                                sq[:, 128 * jj:128 * jj + 128],
                                sel2k_sb[:],
                                start=True, stop=True)
                        lnk = tpool.tile([128, 8], F32, tag="lnk")
                        nc.scalar.activation(lnk[:], rkv[:], AF.Ln,
                                             bias=cc_sb[:, 0:1])
                        nc.scalar.activation(
                            rk_sb[b][:, 4 * tci:4 * tci + 4, :].rearrange(
                                "p a c -> p (a c)"),
                            lnk[:], AF.Exp, scale=-0.5, bias=cc_sb[:, 1:2])
                    else:
                        # v: biased copy, transpose into vhat [tok, d]
                        vsb = tpool.tile([128, CHUNK], BF16, tag="xsb")
                        nc.scalar.activation(vsb[:], mm[:], AF.Identity,
                                             bias=bqkv_sb[:, 2:3])
                        for jj in range(4):
                            blk = 4 * tci + jj
                            tp = ps_st.tile([128, 128], BF16, tag="st")
                            nc.tensor.transpose(
                                tp[:], vsb[:, 128 * jj:128 * jj + 128],
                                identb_sb[:])
                            nc.vector.tensor_copy(
                                vhat[b][:, blk, 0:64], tp[:, 0:64])
                            nc.vector.tensor_copy(
                                vhat[b][:, blk, 65:129], tp[:, 64:128])

            # ================= attention (both heads, one batch) =============
            def attn_pass(b, ps_at, mid=None):
                at = [ps_at.tile([65, 1024], F32, tag=f"at{h}",
                                 name=f"at{b}_{h}")
                      for h in range(HPC)]
                bank_fresh = [[True, True] for _ in range(HPC)]
                for j in range(NBLK):
                    ks = 128 * j
                    w = min(512, S - (ks + 128))  # span width
                    scs = [ps_sc.tile([128, 640], F32, tag="sc",
                                      name=f"sc{h}") for h in range(HPC)]
                    for h in range(HPC):  # spans first: concurrent row groups
                        r0, r1 = 64 * h, 64 * h + 64
                        if w > 0:
                            nc.tensor.matmul(
                                scs[h][:, 0:w],
                                kc[b][r0:r1, ks:ks + 128],
                                qc[b][r0:r1, ks + 128:ks + 128 + w],
                                start=True, stop=True)
                    for h in range(HPC):  # then diagonals
                        r0, r1 = 64 * h, 64 * h + 64
                        nc.tensor.matmul(
                            scs[h][:, 512:640],
                            kc[b][r0:r1, ks:ks + 128],
                            qc[b][r0:r1, ks:ks + 128],
                            start=True, stop=True)
                    exs = []
                    for h in range(HPC):
                        sc = scs[h]
                        ex = epool.tile([128, 640], BF16, tag=f"ex{h}")
                        scale_ap = rk_sb[b][:, j, h:h + 1]
                        if w > 0:
                            nc.scalar.activation(ex[:, :], sc[:, :], AF.Exp,
                                                 scale=scale_ap)
                        else:
                            nc.scalar.activation(ex[:, 512:640],
                                                 sc[:, 512:640],
                                                 AF.Exp, scale=scale_ap)
                        nc.vector.tensor_mul(ex[:, 512:640], ex[:, 512:640],
                                             mlead_sb[:])
                        if w == 512:
                            nc.vector.tensor_mul(ex[:, 384:512],
                                                 ex[:, 384:512], mtrail_sb[:])
                        exs.append(ex)
                    for h in range(HPC):
                        ex = exs[h]
                        segs = []
                        q0 = ks + 128
                        a = q0
                        while a < q0 + w:
                            seglen = min(512 - (a % 512), q0 + w - a)
                            segs.append((a, seglen, a - q0))
                            a += seglen
                        segs.append((ks, 128, 512))  # diagonal last
                        for si, (qstart, qlen, excol) in enumerate(segs):
                            bank = (qstart % 1024) // 512
                            st = bank_fresh[h][bank]
                            bank_fresh[h][bank] = False
                            last = (si == len(segs) - 1) and (j % 4 == 3)
                            nc.tensor.matmul(
                                at[h][:, (qstart % 1024):(qstart % 1024) + qlen],
                                vhat[b][:, j, 65 * h:65 * h + 65],
                                ex[:, excol:excol + qlen],
                                start=st, stop=last)
                    if j % 4 == 3:
                        c = j // 4  # evacuate queries [512c, 512c+512)
                        bank = c % 2
                        cs = 512 * bank
                        slot = b * 4 + c
                        for h in range(HPC):
                            hot = hpool.tile([64, 512], BF16, tag="hot")
                            nc.vector.tensor_copy(hot[:],
                                                  at[h][0:64, cs:cs + 512])
                            rsum = hpool.tile([1, 512], F32, tag="rs")
                            nc.vector.tensor_copy(rsum[:],
                                                  at[h][64:65, cs:cs + 512])
                            ro = 66 * h
                            nc.sync.dma_start(
                                out=partb[b][slot, ro:ro + 64, :], in_=hot[:])
                            nc.sync.dma_start(
                                out=partb[b][slot:slot + 1,
                                             ro + 64:ro + 66, :].rearrange(
                                    "s a c -> s (a c)"),
                                in_=rsum[:].bitcast(BF16))
                            bank_fresh[h][bank] = True
                    if j == 11 and mid is not None:
                        mid()

            # ================= schedule ======================================
            ps_st_cm = tc.tile_pool(name="ps_st", bufs=2, space="PSUM")
            ps_st = ps_st_cm.__enter__()
            for tci in range(CPB):
                proj_chunk(0, tci, ps_st)

            # constants group B (needed from attention onward)
            mlead_sb = cpool.tile([128, 128], BF16)
            nc.sync.dma_start(out=mlead_sb, in_=mlead[:, :])
            mtrail_sb = cpool.tile([128, 128], BF16)
            nc.sync.dma_start(out=mtrail_sb, in_=mtrail[:, :])
            wout_sb = cpool.tile([128, 8, E], BF16)
            nc.sync.dma_start(out=wout_sb,
                              in_=wout[:, :].rearrange("(t p) c -> p t c", p=128))
            bout_sb = cpool.tile([128, 8], F32)
            nc.sync.dma_start(out=bout_sb, in_=bout[:, :])
            sel01_sb = cpool.tile([4, 256], F32R)
            nc.sync.dma_start(out=sel01_sb, in_=sel01[:, :])

            for tci in range(CPB):
                proj_chunk(1, tci, ps_st)

            rcp = ppool.tile([4, 8, 256], F32, tag="rcp", name="rcp")
            rcpr = ppool.tile([4, 8, 256], F32R, tag="rcpr", name="rcpr")

            def stage_loads(x):
                nc.sync.dma_start(
                    out=stg[x][0:64, :, :],
                    in_=a2ab[x][:, 0:64, :].rearrange("h p c -> p h c"))
                nc.sync.dma_start(
                    out=stg[x][64:128, :, :],
                    in_=a2ab[x][:, 66:130, :].rearrange("h p c -> p h c"))
                nc.sync.dma_start(
                    out=rstg[x][0:2, :, :],
                    in_=a2ab[x][:, 64:66, :].rearrange("h a c -> a h c"))
                nc.sync.dma_start(
                    out=rstg[x][2:4, :, :],
                    in_=a2ab[x][:, 130:132, :].rearrange("h a c -> a h c"))
                # 1/(rowsum + tiny): tiny keeps the zero-padded slots finite
                rsg = rstg[x][:].bitcast(F32)
                nc.vector.tensor_scalar_add(rcp[:], rsg, 1e-20)
                nc.vector.reciprocal_approx_fast(out=rcp[:], in_=rcp[:])
                nc.vector.tensor_copy(rcpr[:], rcp[:])

            ps_st_cm.__exit__(None, None, None)
            ps_at_cm = tc.tile_pool(name="ps_at", bufs=1, space="PSUM")
            ps_at = ps_at_cm.__enter__()
            attn_pass(0, ps_at)
            for b in range(B):
                for s in range(4):
                    slot = (1 - b) * 4 + s  # the other batch's slots
                    nc.sync.dma_start(out=partb[b][slot, 0:128, :],
                                      in_=zz_sb[:])
                    nc.sync.dma_start(out=partb[b][slot, 128:132, :],
                                      in_=zz_sb[0:4, :])

            nc.gpsimd.collective_compute(
                "AllToAll",
                mybir.AluOpType.bypass,
                replica_groups=[list(range(N_CORES))],
                ins=[partb[0].opt()],
                outs=[a2ab[0].opt()],
            )
            attn_pass(1, ps_at, mid=lambda: stage_loads(0))
            nc.gpsimd.collective_compute(
                "AllToAll",
                mybir.AluOpType.bypass,
                replica_groups=[list(range(N_CORES))],
                ins=[partb[1].opt()],
                outs=[a2ab[1].opt()],
            )

            ps_at_cm.__exit__(None, None, None)
            ps_ob_cm = tc.tile_pool(name="ps_ob", bufs=4, space="PSUM")
            ps_ob = ps_ob_cm.__enter__()

            # ====== out projection, phase X (X=0 overlaps the 2nd A2A) ======
            osbA = [ppool.tile([128, 512], F32, tag=f"osbA{ot}",
                    name=f"osbA{ot}") for ot in range(8)]
            for x in range(2):
                if x == 1:
                    stage_loads(1)
                for ht in range(8):
                    rbc = ps_ob.tile([128, 512], F32, tag="ost")
                    nc.tensor.matmul(rbc[:, 0:256], sel01_sb[:, 0:128],
                                     rcpr[:, ht, :], start=True, stop=True)
                    nc.tensor.matmul(rbc[:, 256:512], sel01_sb[:, 128:256],
                                     rcpr[:, ht, :], start=True, stop=True)
                    nc.vector.tensor_mul(stg[x][:, ht, :], stg[x][:, ht, :],
                                         rbc[:])
                for ot in range(8):
                    omm = ps_sc.tile([128, 512], F32, tag="sc")
                    for ht in range(8):
                        nc.tensor.matmul(
                            omm[:],
                            wout_sb[:, ht, ot * 128:(ot + 1) * 128],
                            stg[x][:, ht, :],
                            start=(ht == 0), stop=(ht == 7))
                    if x == 0:
                        nc.scalar.activation(osbA[ot][:], omm[:], AF.Identity,
                                             bias=bout_sb[:, ot:ot + 1])
                    else:
                        osb = opool.tile([128, 512], F32, tag="osb")
                        nc.vector.tensor_add(osb[:], omm[:], osbA[ot][:])
                        nc.sync.dma_start(
                            out=outT[ot * 128:(ot + 1) * 128, :], in_=osb[:])
            ps_ob_cm.__exit__(None, None, None)

    _patch_act_tables(nc.m.arch)
    nc.compile()
    return nc


def _make_host_inputs(x, W_qkv, b_qkv, q_gamma, q_beta, k_gamma, k_beta,
                      W_out, b_out):
    assert np.allclose(q_beta, 0.0) and np.allclose(k_beta, 0.0), (
        "kernel only supports beta == 0 qk-norm")
    gp = (np.asarray(q_gamma, np.float32)
          * np.asarray(k_gamma, np.float32))  # [64]
    assert np.all(gp != 0.0), "kernel requires nonzero gamma product"

    bf = ml_dtypes.bfloat16
    xT = np.ascontiguousarray(
        np.transpose(np.asarray(x, np.float32), (2, 0, 1)).reshape(E, TOK)
    ).astype(bf)

    W3 = np.asarray(W_qkv, np.float32).reshape(E, 3, H, D).copy()
    b3 = np.asarray(b_qkv, np.float32).reshape(3, H, D).copy()
    # fold LN mean-centering into the q/k weights and biases
    for c in (0, 1):
        W3[:, c] -= W3[:, c].mean(axis=-1, keepdims=True)
        b3[c] -= b3[c].mean(axis=-1, keepdims=True)
    # fold gamma product into the k side
    W3[:, 1] *= gp[None, None, :]
    b3[1] *= gp[None, :]

    # masks: lead[kj, qi] = (qi >= kj); trail[kj, ci] = (kj > ci)
    kj = np.arange(128)[:, None]
    qi = np.arange(128)[None, :]
    mleadm = (qi >= kj).astype(np.float32)
    mtrailm = (kj > qi).astype(np.float32)

    sel2qm = np.zeros((128, 2), np.float32)
    sel2qm[0:64, 0] = 1.0 / 64.0
    sel2qm[64:128, 1] = 1.0 / 64.0
    sel2km = np.zeros((128, 2), np.float32)
    sel2km[0:64, 0] = 1.0 / (64.0 * gp * gp)
    sel2km[64:128, 1] = 1.0 / (64.0 * gp * gp)
    expdm = np.zeros((2, 128), np.float32)
    expdm[0, 0:64] = 1.0
    expdm[1, 64:128] = 1.0
    identm = np.eye(128, dtype=np.float32)
    # post-a2a denominator broadcast selectors: rcp rows are
    # (h0 tok 0-255, h0 tok 256-511, h1 tok 0-255, h1 tok 256-511)
    sel01m = np.zeros((4, 256), np.float32)
    sel01m[0, 0:64] = 1.0    # sel0: cols 0:128
    sel01m[2, 64:128] = 1.0
    sel01m[1, 128:192] = 1.0  # sel1: cols 128:256
    sel01m[3, 192:256] = 1.0

    woutm = np.ascontiguousarray(np.asarray(W_out, np.float32)).astype(bf)
    boutm = np.ascontiguousarray(
        np.asarray(b_out, np.float32).reshape(8, 128).T)  # [128, 8]

    in_maps = []
    for c in range(N_CORES):
        hsl = slice(HPC * c, HPC * (c + 1))
        wq = W3[:, :, hsl, :].reshape(E, 3 * HPC * D).astype(bf)
        bq = np.ascontiguousarray(
            b3[:, hsl, :].reshape(3, 128).T.astype(np.float32))  # [128, 3]
        in_maps.append({
            "xT": xT,
            "wqkv": np.ascontiguousarray(wq),
            "bqkv": bq,
            "wout": woutm,
            "bout": boutm,
            "mlead": mleadm.astype(bf),
            "mtrail": mtrailm.astype(bf),
            "sel2q": sel2qm.astype(bf),
            "sel2k": sel2km.astype(bf),
            "expd": expdm,
            "identb": identm.astype(bf),
            "sel01": sel01m,
        })
    return in_maps


_CACHED = {}


def _get_program():
    if "nc" not in _CACHED:
        _CACHED["nc"] = build_program()
    return _CACHED["nc"]


def kernel(x, W_qkv, b_qkv, q_gamma, q_beta, k_gamma, k_beta, W_out, b_out,
           _trace=False, **trace_kwargs):
    in_maps = _make_host_inputs(
        x, W_qkv, b_qkv, q_gamma, q_beta, k_gamma, k_beta, W_out, b_out)
    nc = _get_program()
    res = run_bass_kernel_spmd(nc, in_maps, list(range(N_CORES)),
                               trace=_trace, **trace_kwargs)
    outTs = [res.results[c]["outT"] for c in range(N_CORES)]
    full = np.concatenate(outTs, axis=1)  # [E, TOK]
    out = full.reshape(E, B, S).transpose(1, 2, 0)
    if _trace:
        kernel.last_results = res
    return np.ascontiguousarray(out)


if __name__ == "__main__":
    import reference

    inputs = {k: np.asarray(v) for k, v in reference.setup_inputs().items()}
    expected = np.asarray(reference.reference(**inputs))
    actual = kernel(**inputs)
    err = np.abs(actual - expected)
    rel = np.linalg.norm(actual - expected) / np.linalg.norm(expected)
    print("max abs err:", err.max(), "rel fro err:", rel)
